# revision 19
# baseline (speedup 1.0000x reference)
"""Circulant matmul kernel for Trainium2 (8 NeuronCores, SPMD).

Problem: out = input @ K + bias, where K[c, n] = weight[(c - n) mod 4096],
input is [1024, 4096] f32, weight/bias are [4096] f32.

Strategy (tensor-parallel / column-shard, per the sharding hint):
  - Host materializes X^T in bf16 (replicated to all 8 cores) and each
    core's 512-column slice of the circulant matrix K in bf16.
  - Core c computes out[:, 512c:512(c+1)] = X @ K_c + bias_c in fp32 PSUM.
    No collectives; host concatenates the 8 column slices.

Device kernel structure (per core):
  - xt chunks (32 x [128, 1024] bf16) DMA'd on the sync HWDGE queue,
    kc chunks (32 x [128, 512] bf16) on the scalar HWDGE queue, so
    descriptor generation is parallelized across both HW-DGE rings.
  - PE warm-up: full-width dummy matmuls on a DVE-memset scratch tile
    while the first input chunks land (lifts the HAM clock gate early).
  - Phase 1 processes chunks 0..23 across all 8 batch tiles (co-major,
    matches DMA arrival); phase 2 finishes each batch tile in turn
    (bt-major) so the bias-add + output DMA epilogues overlap the
    remaining matmuls.
"""

import numpy as np
import ml_dtypes

import concourse.bass as bass
import concourse.mybir as mybir
import concourse.tile as tile
from concourse import bacc
from concourse.bass import ts
from concourse.bass_utils import run_bass_kernel_spmd

N = 4096
BATCH = 1024
NCORES = 8
NSHARD = N // NCORES          # 512 output columns per core
P = 128                       # partitions
CO = N // P                   # 32 contraction chunks
BT = BATCH // P               # 8 batch tiles
CO_PH1 = CO - BT              # chunks processed co-major in phase 1

N_WARMUP = 9                  # full-width dummy matmuls to lift the HAM clock gate

BF16 = mybir.dt.bfloat16
F32 = mybir.dt.float32


def build_nc():
    """Build the per-core Bass program (same program on all cores; data differs)."""
    nc = bacc.Bacc("TRN2", target_bir_lowering=False, debug=False)

    xt_d = nc.dram_tensor("xt", [N, BATCH], BF16, kind="ExternalInput").ap()
    kc_d = nc.dram_tensor("kc", [N, NSHARD], BF16, kind="ExternalInput").ap()
    bias_d = nc.dram_tensor("biasb", [P, NSHARD], F32, kind="ExternalInput").ap()
    out_d = nc.dram_tensor("out", [BATCH, NSHARD], BF16, kind="ExternalOutput").ap()

    xt_r = xt_d.rearrange("(co ci) b -> ci co b", ci=P)      # [128, 32, 1024]
    kc_r = kc_d.rearrange("(co ci) n -> ci co n", ci=P)      # [128, 32, 512]

    with tile.TileContext(nc) as tc:
        with (
            tc.tile_pool(name="xpool", bufs=CO) as xpool,
            tc.tile_pool(name="kpool", bufs=CO) as kpool,
            tc.tile_pool(name="cpool", bufs=1) as cpool,
            tc.tile_pool(name="opool", bufs=4) as opool,
            tc.tile_pool(name="psum", bufs=BT, space="PSUM") as psum_pool,
        ):
            # priming DMAs: 16 descriptor rows on each HWDGE ring wake all
            # 16 SDMA engines so the first real chunk transfers at full
            # rate instead of the ~1us cold ramp
            px = cpool.tile([16, 256], BF16, tag="px")
            nc.sync.dma_start(px[:], xt_r[:16, 0, :256])
            pk = cpool.tile([16, 256], BF16, tag="pk")
            nc.scalar.dma_start(pk[:], kc_r[:16, 0, :256])

            # scratch for PE warm-up, memset on the vector engine (fast start)
            scratch = cpool.tile([P, NSHARD], BF16, tag="scratch")
            nc.vector.memset(scratch[:], 0.125)

            # input streams: kc on scalar ring, xt on sync ring
            xt_tiles = []
            kc_tiles = []
            for co in range(CO):
                ktt = kpool.tile([P, NSHARD], BF16, tag="kc")
                nc.scalar.dma_start(ktt[:], kc_r[:, co, :])
                kc_tiles.append(ktt)
                xtt = xpool.tile([P, BATCH], BF16, tag="xt")
                nc.sync.dma_start(xtt[:], xt_r[:, co, :])
                xt_tiles.append(xtt)
            # bias last on the scalar ring: only needed for the epilogues
            bias_sb = cpool.tile([P, NSHARD], F32, tag="bias")
            nc.scalar.dma_start(bias_sb[:], bias_d)

            psum_tiles = [
                psum_pool.tile([P, NSHARD], F32, tag="ps", name=f"ps{bt}")
                for bt in range(BT)
            ]

            # PE warm-up: full 128-row dummy matmuls on scratch (HAM needs
            # real array activity; results are discarded by start=True below)
            for i in range(N_WARMUP):
                nc.tensor.matmul(
                    psum_tiles[i % BT][:],
                    scratch[:, :P],
                    scratch[:],
                    start=True,
                    stop=True,
                )

            # phase 1: chunks 0..CO_PH1-1, co-major (matches DMA arrival order)
            for co in range(CO_PH1):
                for bt in range(BT):
                    nc.tensor.matmul(
                        psum_tiles[bt][:],
                        xt_tiles[co][:, ts(bt, P)],   # lhsT [c=128, b=128]
                        kc_tiles[co][:],              # rhs  [c=128, n=512]
                        start=(co == 0),
                        stop=False,
                    )

            # phase 2: finish batch tiles one at a time; epilogue overlaps MMs
            for bt in range(BT):
                for co in range(CO_PH1, CO):
                    nc.tensor.matmul(
                        psum_tiles[bt][:],
                        xt_tiles[co][:, ts(bt, P)],
                        kc_tiles[co][:],
                        start=False,
                        stop=(co == CO - 1),
                    )
                out_sb = opool.tile([P, NSHARD], BF16, tag="osb")
                nc.vector.tensor_add(out_sb[:], psum_tiles[bt][:], bias_sb[:])
                nc.sync.dma_start(out_d[ts(bt, P), :], out_sb[:])

    nc.compile()
    return nc


def prepare_in_maps(input, weight, bias):
    x = np.asarray(input, dtype=np.float32)
    w = np.asarray(weight, dtype=np.float32)
    b = np.asarray(bias, dtype=np.float32)

    xt = np.ascontiguousarray(x.T).astype(ml_dtypes.bfloat16)   # [4096, 1024]

    c = np.arange(N)
    in_maps = []
    for core in range(NCORES):
        n0 = core * NSHARD
        idx = (c[:, None] - (n0 + np.arange(NSHARD))[None, :]) % N
        kc = w[idx].astype(ml_dtypes.bfloat16)                  # [4096, 512]
        bias_tile = np.ascontiguousarray(
            np.broadcast_to(b[n0 : n0 + NSHARD].astype(np.float32), (P, NSHARD))
        )
        in_maps.append({"xt": xt, "kc": kc, "biasb": bias_tile})
    return in_maps


_NC_CACHE = None


def _get_nc():
    global _NC_CACHE
    if _NC_CACHE is None:
        _NC_CACHE = build_nc()
    return _NC_CACHE


def kernel(**inputs):
    nc = _get_nc()
    in_maps = prepare_in_maps(inputs["input"], inputs["weight"], inputs["bias"])
    res = run_bass_kernel_spmd(nc, in_maps, list(range(NCORES)))
    out = np.empty((BATCH, N), dtype=np.float32)
    for core in range(NCORES):
        out[:, core * NSHARD : (core + 1) * NSHARD] = res.results[core]["out"].astype(
            np.float32
        )
    return out


# revision 20
# speedup vs baseline: 1.0438x; 1.0438x over previous
"""Circulant matmul kernel for Trainium2 (8 NeuronCores, SPMD).

Problem: out = input @ K + bias, where K[c, n] = weight[(c - n) mod 4096],
input is [1024, 4096] f32, weight/bias are [4096] f32.

Strategy (tensor-parallel / column-shard, per the sharding hint):
  - Host materializes X^T in bf16 (replicated to all 8 cores) and each
    core's 512-column slice of the circulant matrix K in bf16.
  - Core c computes out[:, 512c:512(c+1)] = X @ K_c + bias_c in fp32 PSUM.
    No collectives; host concatenates the 8 column slices.

Device kernel structure (per core):
  - xt chunks (32 x [128, 1024] bf16) DMA'd on the sync HWDGE queue,
    kc chunks (32 x [128, 512] bf16) on the scalar HWDGE queue, so
    descriptor generation is parallelized across both HW-DGE rings.
  - PE warm-up: full-width dummy matmuls on a DVE-memset scratch tile
    while the first input chunks land (lifts the HAM clock gate early).
  - Phase 1 processes chunks 0..23 across all 8 batch tiles (co-major,
    matches DMA arrival); phase 2 finishes each batch tile in turn
    (bt-major) so the bias-add + output DMA epilogues overlap the
    remaining matmuls.
"""

import numpy as np
import ml_dtypes

import concourse.bass as bass
import concourse.mybir as mybir
import concourse.tile as tile
from concourse import bacc
from concourse.bass import ts
from concourse.bass_utils import run_bass_kernel_spmd

N = 4096
BATCH = 1024
NCORES = 8
NSHARD = N // NCORES          # 512 output columns per core
P = 128                       # partitions
CO = N // P                   # 32 contraction chunks
BT = BATCH // P               # 8 batch tiles
CO_PH1 = CO - BT              # chunks processed co-major in phase 1

N_WARMUP = 9                  # full-width dummy matmuls to lift the HAM clock gate

BF16 = mybir.dt.bfloat16
F32 = mybir.dt.float32


def build_nc():
    """Build the per-core Bass program (same program on all cores; data differs)."""
    nc = bacc.Bacc("TRN2", target_bir_lowering=False, debug=False)

    xt_d = nc.dram_tensor("xt", [N, BATCH], BF16, kind="ExternalInput").ap()
    kc_d = nc.dram_tensor("kc", [N, NSHARD], BF16, kind="ExternalInput").ap()
    bias_d = nc.dram_tensor("biasb", [P, NSHARD], F32, kind="ExternalInput").ap()
    out_d = nc.dram_tensor("out", [BATCH, NSHARD], BF16, kind="ExternalOutput").ap()

    xt_r = xt_d.rearrange("(co ci) b -> ci co b", ci=P)      # [128, 32, 1024]
    kc_r = kc_d.rearrange("(co ci) n -> ci co n", ci=P)      # [128, 32, 512]

    with tile.TileContext(nc) as tc:
        with (
            tc.tile_pool(name="xpool", bufs=CO) as xpool,
            tc.tile_pool(name="kpool", bufs=CO) as kpool,
            tc.tile_pool(name="cpool", bufs=1) as cpool,
            tc.tile_pool(name="opool", bufs=4) as opool,
            tc.tile_pool(name="psum", bufs=BT, space="PSUM") as psum_pool,
        ):
            # scratch for PE warm-up, memset on the vector engine (fast start)
            scratch = cpool.tile([P, NSHARD], BF16, tag="scratch")
            nc.vector.memset(scratch[:], 0.125)

            # input streams: kc on scalar ring, xt on sync ring
            xt_tiles = []
            kc_tiles = []
            for co in range(CO):
                ktt = kpool.tile([P, NSHARD], BF16, tag="kc")
                nc.scalar.dma_start(ktt[:], kc_r[:, co, :])
                kc_tiles.append(ktt)
                xtt = xpool.tile([P, BATCH], BF16, tag="xt")
                nc.sync.dma_start(xtt[:], xt_r[:, co, :])
                xt_tiles.append(xtt)
            # bias last on the scalar ring: only needed for the epilogues
            bias_sb = cpool.tile([P, NSHARD], F32, tag="bias")
            nc.scalar.dma_start(bias_sb[:], bias_d)

            psum_tiles = [
                psum_pool.tile([P, NSHARD], F32, tag="ps", name=f"ps{bt}")
                for bt in range(BT)
            ]

            # PE warm-up: full 128-row dummy matmuls on scratch (HAM needs
            # real array activity; results are discarded by start=True below)
            for i in range(N_WARMUP):
                nc.tensor.matmul(
                    psum_tiles[i % BT][:],
                    scratch[:, :P],
                    scratch[:],
                    start=True,
                    stop=True,
                )

            # phase 1: chunks 0..CO_PH1-1, co-major (matches DMA arrival order)
            for co in range(CO_PH1):
                for bt in range(BT):
                    nc.tensor.matmul(
                        psum_tiles[bt][:],
                        xt_tiles[co][:, ts(bt, P)],   # lhsT [c=128, b=128]
                        kc_tiles[co][:],              # rhs  [c=128, n=512]
                        start=(co == 0),
                        stop=False,
                    )

            # phase 2: finish batch tiles one at a time; epilogue overlaps MMs
            for bt in range(BT):
                for co in range(CO_PH1, CO):
                    nc.tensor.matmul(
                        psum_tiles[bt][:],
                        xt_tiles[co][:, ts(bt, P)],
                        kc_tiles[co][:],
                        start=False,
                        stop=(co == CO - 1),
                    )
                out_sb = opool.tile([P, NSHARD], BF16, tag="osb")
                nc.vector.tensor_add(out_sb[:], psum_tiles[bt][:], bias_sb[:])
                nc.sync.dma_start(out_d[ts(bt, P), :], out_sb[:])

    nc.compile()
    return nc


def prepare_in_maps(input, weight, bias):
    x = np.asarray(input, dtype=np.float32)
    w = np.asarray(weight, dtype=np.float32)
    b = np.asarray(bias, dtype=np.float32)

    xt = np.ascontiguousarray(x.T).astype(ml_dtypes.bfloat16)   # [4096, 1024]

    c = np.arange(N)
    in_maps = []
    for core in range(NCORES):
        n0 = core * NSHARD
        idx = (c[:, None] - (n0 + np.arange(NSHARD))[None, :]) % N
        kc = w[idx].astype(ml_dtypes.bfloat16)                  # [4096, 512]
        bias_tile = np.ascontiguousarray(
            np.broadcast_to(b[n0 : n0 + NSHARD].astype(np.float32), (P, NSHARD))
        )
        in_maps.append({"xt": xt, "kc": kc, "biasb": bias_tile})
    return in_maps


_NC_CACHE = None


def _get_nc():
    global _NC_CACHE
    if _NC_CACHE is None:
        _NC_CACHE = build_nc()
    return _NC_CACHE


def kernel(**inputs):
    nc = _get_nc()
    in_maps = prepare_in_maps(inputs["input"], inputs["weight"], inputs["bias"])
    res = run_bass_kernel_spmd(nc, in_maps, list(range(NCORES)))
    out = np.empty((BATCH, N), dtype=np.float32)
    for core in range(NCORES):
        out[:, core * NSHARD : (core + 1) * NSHARD] = res.results[core]["out"].astype(
            np.float32
        )
    return out
